# revision 23
# baseline (speedup 1.0000x reference)
"""Trainium2 Bass kernel for nn_MultiHeadAttention (B=2, S=4096, D=512, H=8).

Sharding: sequence-parallel over queries. 8 cores = 2 batches x 4 query
quarters of 1024 rows each. Each core holds the full (mask-compacted) K/V
of its batch, computes its query rows end-to-end (Q/K/V projections,
masked softmax attention, output projection), and writes its disjoint
output rows. Host concatenates - no collectives needed.

Mask handling: the mask is a key-padding mask (per batch, per key).
Masked keys contribute exactly zero to softmax numerator and denominator,
so we compact them away on the host (halves all attention work; the
result is mathematically identical). Padding rows up to a multiple of
128 get a -30 additive bias so exp() sends them to ~1e-13.

Device dataflow (per core, SQ=1024 query rows, SK ~= 2176 keys):
  QT[o,q]  = WqT.T @ xqT     (bf16 matmuls, fp32 PSUM accumulation)
  KT[o,k]  = WkT.T @ xkT
  V[k,o]   = xvT.T @ WvT     -> packed as Vpad[k][h][V_h(64) | ones(64)]
  per head h:
    S^T[k,q] = KT_h.T(aka K_h) @ QT_h      (k-chunks of 128)
    E[k,q]   = exp(0.125*S^T + bias[k])    (ScalarE, bf16 out)
    PV[128,q] accum= Vpad_h.T @ E          rows 0:64 = numerator^T,
                                           rows 64:128 = denominator (x64)
    OnT[h]   = numerator^T * 1/denominator (bf16)
  out[q,j] = sum_h OnT_h.T @ WoT_h  (fp32) -> DMA to DRAM
"""

import numpy as np
import ml_dtypes

B, S, D, H, DK = 2, 4096, 512, 8, 64
NCORES = 8
QSH = 4          # query shards per batch
SQ = S // QSH    # 1024 query rows per core

BF16 = ml_dtypes.bfloat16

_BUILD_CACHE = {}
LAST_RESULTS = None
LAST_IN_MAPS = None


def _build(KC):
    """Build the Bass/Tile program for SK = KC*128 compacted+padded keys."""
    from contextlib import ExitStack

    import concourse.mybir as mybir
    import concourse.tile as tile
    from concourse import bacc

    SK = KC * 128
    f32 = mybir.dt.float32
    bf16 = mybir.dt.bfloat16

    nc = bacc.Bacc(
        "TRN2",
        target_bir_lowering=False,
        debug=False,
        enable_asserts=False,
        num_devices=NCORES,
    )

    def din(name, shape, dt):
        return nc.dram_tensor(name, shape, dt, kind="ExternalInput").ap()

    d_xqT = din("xqT", [128, 4, SQ], bf16)
    d_xkT = din("xkT", [128, 4, SK], bf16)
    d_xvT = din("xvT", [128, 4, SK], bf16)
    d_bias = din("bias", [128, KC], f32)
    d_wqT = din("wqT", [128, 4, D], bf16)
    d_wkT = din("wkT", [128, 4, D], bf16)
    d_wvT = din("wvT", [128, 4, D], bf16)
    d_woT = din("woT", [128, 4, D], bf16)
    d_out = nc.dram_tensor("out", [SQ, D], f32, kind="ExternalOutput").ap()

    Exp = mybir.ActivationFunctionType.Exp
    mult = mybir.AluOpType.mult

    def nslices(total, step=512):
        return [(s, min(step, total - s)) for s in range(0, total, step)]

    with tile.TileContext(nc) as tc:
        with ExitStack() as ctx:
            sb = ctx.enter_context(tc.tile_pool(name="sb", bufs=1))

            # ---- persistent SBUF tensors ----
            t_xqT = sb.tile([128, 4, SQ], bf16, tag="xqT")
            t_xkT = sb.tile([128, 4, SK], bf16, tag="xkT")
            t_xvT = sb.tile([128, 4, SK], bf16, tag="xvT")
            t_bias = sb.tile([128, KC], f32, tag="bias")
            t_wqT = sb.tile([128, 4, D], bf16, tag="wqT")
            t_wkT = sb.tile([128, 4, D], bf16, tag="wkT")
            t_wvT = sb.tile([128, 4, D], bf16, tag="wvT")
            t_woT = sb.tile([128, 4, D], bf16, tag="woT")
            t_QT = sb.tile([128, 4, SQ], bf16, tag="QT")
            t_KT = sb.tile([128, 4, SK], bf16, tag="KT")
            # Vpad[k, kc, h, 0:64] = V_h rows, [.., 64:128] = 1.0 (denominator)
            t_V = sb.tile([128, KC, H, 128], bf16, tag="V")
            # normalized attention out, head-PAIR packed: head 2c on
            # partitions 0:63, head 2c+1 on 64:127 (via DMA) -> K=128 final
            t_OnT = sb.tile([128, 4, SQ], bf16, tag="OnT")
            # running output accumulator: out[q, :] = sum_c OnT_c.T @ WoT_c,
            # accumulated per head-pair so no serial tail projection remains
            t_oacc = sb.tile([128, SQ // 128, D], f32, tag="oacc")

            # Load order: first-needed first. A single dma_start tops out
            # around ~90 GB/s, so split the big tensors into per-ic chunks
            # across three DMA-issuing engines (SP/ACT HWDGE + Pool SWDGE;
            # ACT and Pool are idle at t=0) to run closer to HBM bandwidth.
            for ic in range(4):
                nc.sync.dma_start(t_wqT[:, ic, :], d_wqT[:, ic, :])
                nc.sync.dma_start(t_xqT[:, ic, :], d_xqT[:, ic, :])
                nc.scalar.dma_start(t_wkT[:, ic, :], d_wkT[:, ic, :])
                nc.scalar.dma_start(t_xkT[:, ic, :], d_xkT[:, ic, :])
            nc.gpsimd.dma_start(t_wvT[:], d_wvT)
            for ic in range(4):
                nc.sync.dma_start(t_xvT[:, ic, :], d_xvT[:, ic, :])
            nc.gpsimd.dma_start(t_bias[:], d_bias)
            nc.gpsimd.dma_start(t_woT[:], d_woT)

            nc.vector.memset(t_V[:, :, :, 64:128], 1.0)

            # PSUM budget (8 banks): head pairs run concurrently on PE
            # row-groups 0:63 / 64:127. Tags:
            #   pssa/pssb x1 buf = 4 banks (fp32 scores + projection psums)
            #   ppva/ppvb x1 buf = 4 banks (PV accumulators, fp32)
            ps_pool = ctx.enter_context(
                tc.tile_pool(name="ps_s", bufs=1, space="PSUM"))
            pv_pool = ctx.enter_context(
                tc.tile_pool(name="ps_pv", bufs=1, space="PSUM"))
            ep = ctx.enter_context(tc.tile_pool(name="ep", bufs=2))
            rp = ctx.enter_context(tc.tile_pool(name="rp", bufs=2))
            ob_pool = ctx.enter_context(tc.tile_pool(name="ob", bufs=3))

            # ---- phase 1: projections (ordered so heads 0/1 unblock first)
            _ptag = [0]

            def _proj_ps():
                _ptag[0] ^= 1
                return ps_pool.tile([128, 512], f32, name="psproj",
                                    tag="pssa" if _ptag[0] else "pssb")

            def proj_q(oc):
                for qs, qn in nslices(SQ):
                    ps = _proj_ps()
                    for ic in range(4):
                        nc.tensor.matmul(
                            ps[:, :qn],
                            t_wqT[:, ic, oc * 128:(oc + 1) * 128],
                            t_xqT[:, ic, qs:qs + qn],
                            start=(ic == 0),
                            stop=(ic == 3),
                        )
                    nc.vector.tensor_copy(t_QT[:, oc, qs:qs + qn], ps[:, :qn])

            def proj_k(oc):
                for ks, kn in nslices(SK):
                    ps = _proj_ps()
                    for ic in range(4):
                        nc.tensor.matmul(
                            ps[:, :kn],
                            t_wkT[:, ic, oc * 128:(oc + 1) * 128],
                            t_xkT[:, ic, ks:ks + kn],
                            start=(ic == 0),
                            stop=(ic == 3),
                        )
                    nc.vector.tensor_copy(t_KT[:, oc, ks:ks + kn], ps[:, :kn])

            proj_q(0)
            proj_k(0)
            for sc in range(KC):
                ps = _proj_ps()
                for ic in range(4):
                    nc.tensor.matmul(
                        ps[:],
                        t_xvT[:, ic, sc * 128:(sc + 1) * 128],
                        t_wvT[:, ic, :],
                        start=(ic == 0),
                        stop=(ic == 3),
                    )
                nc.vector.tensor_copy(
                    t_V[:, sc, :, 0:64],
                    ps.rearrange("p (h d) -> p h d", h=H),
                )
            for oc in range(1, 4):
                proj_q(oc)
                proj_k(oc)

            # ---- phase 2: attention (head pairs on PE row groups) ----
            for hc in range(4):
                ppv = {0: pv_pool.tile([128, SQ], f32, name="ppva", tag="ppva"),
                       1: pv_pool.tile([128, SQ], f32, name="ppvb", tag="ppvb")}
                for kc in range(KC):
                    es = {}
                    for hp in (0, 1):
                        pss = ps_pool.tile([128, SQ], f32, name="pss",
                                           tag="pssa" if hp == 0 else "pssb")
                        for qs, qn in nslices(SQ):
                            nc.tensor.matmul(
                                pss[:, qs:qs + qn],
                                t_KT[hp * 64:(hp + 1) * 64, hc,
                                     kc * 128:(kc + 1) * 128],
                                t_QT[hp * 64:(hp + 1) * 64, hc, qs:qs + qn],
                                start=True,
                                stop=True,
                            )
                        e = ep.tile([128, SQ], bf16, name="e",
                                    tag="ea" if hp == 0 else "eb")
                        nc.scalar.activation(
                            e[:], pss[:], Exp,
                            bias=t_bias[:, kc:kc + 1], scale=0.125,
                        )
                        es[hp] = e
                    for hp in (0, 1):
                        for qs, qn in nslices(SQ):
                            nc.tensor.matmul(
                                ppv[hp][:, qs:qs + qn],
                                t_V[:, kc, 2 * hc + hp, :],
                                es[hp][:, qs:qs + qn],
                                start=(kc == 0),
                                stop=(kc == KC - 1),
                            )
                for hp in (0, 1):
                    # Evacuate num+denom PSUM to SBUF in one copy (frees the
                    # PV accumulator for the next pair ASAP). DVE lanes are
                    # partition-locked and the custom-DVE reciprocal only
                    # works at base partition 0, so DMA the denominator from
                    # partitions 64:127 down to 0:63 before inverting.
                    pv_sb = rp.tile([128, SQ], f32, tag="pvsb")
                    den_lo = rp.tile([64, SQ], f32, tag="denlo")
                    rc_lo = rp.tile([64, SQ], f32, tag="rcl")
                    nc.vector.tensor_copy(pv_sb[:], ppv[hp][:])
                    nc.sync.dma_start(den_lo[:], pv_sb[64:128, :])
                    nc.vector.reciprocal_approx_fast(rc_lo[:], den_lo[:])
                    if hp == 0:
                        nc.vector.tensor_tensor(
                            t_OnT[0:64, hc, :], pv_sb[0:64, :], rc_lo[:], mult
                        )
                    else:
                        ot = rp.tile([64, SQ], bf16, tag="ottmp")
                        nc.vector.tensor_tensor(
                            ot[:], pv_sb[0:64, :], rc_lo[:], mult
                        )
                        nc.sync.dma_start(t_OnT[64:128, hc, :], ot[:])

                # ---- per-pair output projection: accumulate this pair's
                # contribution out_qc += OnT_hc.T @ WoT_hc so only the last
                # pair's work remains after attention finishes.
                for qc in range(SQ // 128):
                    po = _proj_ps()
                    nc.tensor.matmul(
                        po[:],
                        t_OnT[:, hc, qc * 128:(qc + 1) * 128],
                        t_woT[:, hc, :],
                        start=True,
                        stop=True,
                    )
                    if hc == 0:
                        nc.vector.tensor_copy(t_oacc[:, qc, :], po[:])
                    elif hc < 3:
                        nc.vector.tensor_add(
                            t_oacc[:, qc, :], t_oacc[:, qc, :], po[:]
                        )
                    else:
                        ob = ob_pool.tile([128, 512], f32, tag="ob")
                        nc.vector.tensor_add(ob[:], t_oacc[:, qc, :], po[:])
                        nc.sync.dma_start(
                            d_out[qc * 128:(qc + 1) * 128, :], ob[:]
                        )

    nc.finalize()
    return nc


def _pack_T(x):
    """[n, 512] fp32 -> transposed bf16 packed [128, 4, n] (contiguous)."""
    n = x.shape[0]
    return np.ascontiguousarray(
        x.T.astype(BF16).reshape(4, 128, n).transpose(1, 0, 2)
    )


def kernel(query, key, value, mask, W_q, W_k, W_v, W_o):
    global LAST_RESULTS, LAST_IN_MAPS
    from concourse.bass_utils import run_bass_kernel_spmd

    query = np.asarray(query, np.float32)
    key = np.asarray(key, np.float32)
    value = np.asarray(value, np.float32)
    mask = np.asarray(mask)

    # -- host prep: mask compaction, transposes, bf16 casts, packing --
    sels = [np.nonzero(mask[b, 0, 0] != 0)[0] for b in range(B)]
    SK = ((max(len(s) for s in sels) + 127) // 128) * 128
    KC = SK // 128

    per_batch = []
    for b in range(B):
        sel = sels[b]
        nk = len(sel)
        xk = np.zeros((SK, D), np.float32)
        xk[:nk] = key[b][sel]
        xv = np.zeros((SK, D), np.float32)
        xv[:nk] = value[b][sel]
        bias = np.full(SK, -30.0, np.float32)
        bias[:nk] = 0.0
        per_batch.append({
            "xkT": _pack_T(xk),
            "xvT": _pack_T(xv),
            "bias": np.ascontiguousarray(bias.reshape(KC, 128).T),
        })

    wqT = _pack_T(np.asarray(W_q, np.float32))   # W_q.T packed: [128,4,512]
    wkT = _pack_T(np.asarray(W_k, np.float32))
    wvT = _pack_T(np.asarray(W_v, np.float32))
    woT = _pack_T(np.asarray(W_o, np.float32))  # [128, 4, 512], head-pair rows

    in_maps = []
    for c in range(NCORES):
        b, qc = divmod(c, QSH)
        xq = query[b, qc * SQ:(qc + 1) * SQ]
        in_maps.append({
            "xqT": _pack_T(xq),
            "xkT": per_batch[b]["xkT"],
            "xvT": per_batch[b]["xvT"],
            "bias": per_batch[b]["bias"],
            "wqT": wqT, "wkT": wkT, "wvT": wvT, "woT": woT,
        })

    if KC not in _BUILD_CACHE:
        _BUILD_CACHE[KC] = _build(KC)
    nc = _BUILD_CACHE[KC]

    LAST_IN_MAPS = in_maps
    res = run_bass_kernel_spmd(nc, in_maps, core_ids=list(range(NCORES)))
    LAST_RESULTS = res

    out = np.empty((B, S, D), np.float32)
    for c in range(NCORES):
        b, qc = divmod(c, QSH)
        out[b, qc * SQ:(qc + 1) * SQ] = res.results[c]["out"]
    return out


# revision 24
# speedup vs baseline: 1.2287x; 1.2287x over previous
"""Trainium2 Bass kernel for nn_MultiHeadAttention (B=2, S=4096, D=512, H=8).

Sharding: sequence-parallel over queries. 8 cores = 2 batches x 4 query
quarters of 1024 rows each. Each core holds the full (mask-compacted) K/V
of its batch, computes its query rows end-to-end (Q/K/V projections,
masked softmax attention, output projection), and writes its disjoint
output rows. Host concatenates - no collectives needed.

Mask handling: the mask is a key-padding mask (per batch, per key).
Masked keys contribute exactly zero to softmax numerator and denominator,
so we compact them away on the host (halves all attention work; the
result is mathematically identical). Padding rows up to a multiple of
128 get a -30 additive bias so exp() sends them to ~1e-13.

Device dataflow (per core, SQ=1024 query rows, SK ~= 2176 keys):
  QT[o,q]  = WqT.T @ xqT     (bf16 matmuls, fp32 PSUM accumulation)
  KT[o,k]  = WkT.T @ xkT
  V[k,o]   = xvT.T @ WvT     -> packed as Vpad[k][h][V_h(64) | ones(64)]
  per head h:
    S^T[k,q] = KT_h.T(aka K_h) @ QT_h      (k-chunks of 128)
    E[k,q]   = exp(0.125*S^T + bias[k])    (ScalarE, bf16 out)
    PV[128,q] accum= Vpad_h.T @ E          rows 0:64 = numerator^T,
                                           rows 64:128 = denominator (x64)
    OnT[h]   = numerator^T * 1/denominator (bf16)
  out[q,j] = sum_h OnT_h.T @ WoT_h  (fp32) -> DMA to DRAM
"""

import numpy as np
import ml_dtypes

B, S, D, H, DK = 2, 4096, 512, 8, 64
NCORES = 8
QSH = 4          # query shards per batch
SQ = S // QSH    # 1024 query rows per core

BF16 = ml_dtypes.bfloat16

_BUILD_CACHE = {}
LAST_RESULTS = None
LAST_IN_MAPS = None


def _build(KC):
    """Build the Bass/Tile program for SK = KC*128 compacted+padded keys."""
    from contextlib import ExitStack

    import concourse.mybir as mybir
    import concourse.tile as tile
    from concourse import bacc

    SK = KC * 128
    f32 = mybir.dt.float32
    bf16 = mybir.dt.bfloat16

    nc = bacc.Bacc(
        "TRN2",
        target_bir_lowering=False,
        debug=False,
        enable_asserts=False,
        num_devices=NCORES,
    )

    def din(name, shape, dt):
        return nc.dram_tensor(name, shape, dt, kind="ExternalInput").ap()

    d_xqT = din("xqT", [128, 4, SQ], bf16)
    d_xkT = din("xkT", [128, 4, SK], bf16)
    d_xvT = din("xvT", [128, 4, SK], bf16)
    d_bias = din("bias", [128, KC], f32)
    d_wqT = din("wqT", [128, 4, D], bf16)
    d_wkT = din("wkT", [128, 4, D], bf16)
    d_wvT = din("wvT", [128, 4, D], bf16)
    d_woT = din("woT", [128, 4, D], bf16)
    d_out = nc.dram_tensor("out", [SQ, D], f32, kind="ExternalOutput").ap()

    Exp = mybir.ActivationFunctionType.Exp
    mult = mybir.AluOpType.mult

    def nslices(total, step=512):
        return [(s, min(step, total - s)) for s in range(0, total, step)]

    with tile.TileContext(nc) as tc:
        with ExitStack() as ctx:
            sb = ctx.enter_context(tc.tile_pool(name="sb", bufs=1))

            # ---- persistent SBUF tensors ----
            t_xqT = sb.tile([128, 4, SQ], bf16, tag="xqT")
            t_xkT = sb.tile([128, 4, SK], bf16, tag="xkT")
            t_xvT = sb.tile([128, 4, SK], bf16, tag="xvT")
            t_bias = sb.tile([128, KC], f32, tag="bias")
            t_wqT = sb.tile([128, 4, D], bf16, tag="wqT")
            t_wkT = sb.tile([128, 4, D], bf16, tag="wkT")
            t_wvT = sb.tile([128, 4, D], bf16, tag="wvT")
            t_woT = sb.tile([128, 4, D], bf16, tag="woT")
            t_QT = sb.tile([128, 4, SQ], bf16, tag="QT")
            t_KT = sb.tile([128, 4, SK], bf16, tag="KT")
            # Vpad[k, kc, h, 0:64] = V_h rows, [.., 64:128] = 1.0 (denominator)
            t_V = sb.tile([128, KC, H, 128], bf16, tag="V")
            # normalized attention out, head-PAIR packed: head 2c on
            # partitions 0:63, head 2c+1 on 64:127 (via DMA) -> K=128 final
            t_OnT = sb.tile([128, 4, SQ], bf16, tag="OnT")

            # Load order: first-needed first. A single dma_start tops out
            # around ~90 GB/s, so split the big tensors into per-ic chunks
            # across three DMA-issuing engines (SP/ACT HWDGE + Pool SWDGE;
            # ACT and Pool are idle at t=0) to run closer to HBM bandwidth.
            for ic in range(4):
                nc.sync.dma_start(t_wqT[:, ic, :], d_wqT[:, ic, :])
                nc.sync.dma_start(t_xqT[:, ic, :], d_xqT[:, ic, :])
                nc.scalar.dma_start(t_wkT[:, ic, :], d_wkT[:, ic, :])
                nc.scalar.dma_start(t_xkT[:, ic, :], d_xkT[:, ic, :])
            nc.gpsimd.dma_start(t_wvT[:], d_wvT)
            for ic in range(4):
                nc.sync.dma_start(t_xvT[:, ic, :], d_xvT[:, ic, :])
            nc.gpsimd.dma_start(t_bias[:], d_bias)
            nc.gpsimd.dma_start(t_woT[:], d_woT)

            nc.vector.memset(t_V[:, :, :, 64:128], 1.0)

            # PSUM budget (8 banks): head pairs run concurrently on PE
            # row-groups 0:63 / 64:127. Tags:
            #   pssa/pssb x1 buf = 4 banks (fp32 scores + projection psums)
            #   ppva/ppvb x1 buf = 4 banks (PV accumulators, fp32)
            ps_pool = ctx.enter_context(
                tc.tile_pool(name="ps_s", bufs=1, space="PSUM"))
            pv_pool = ctx.enter_context(
                tc.tile_pool(name="ps_pv", bufs=1, space="PSUM"))
            ep = ctx.enter_context(tc.tile_pool(name="ep", bufs=2))
            rp = ctx.enter_context(tc.tile_pool(name="rp", bufs=2))
            ob_pool = ctx.enter_context(tc.tile_pool(name="ob", bufs=3))

            # ---- phase 1: projections (ordered so heads 0/1 unblock first)
            _ptag = [0]

            def _proj_ps():
                _ptag[0] ^= 1
                return ps_pool.tile([128, 512], f32, name="psproj",
                                    tag="pssa" if _ptag[0] else "pssb")

            def proj_q(oc):
                for qs, qn in nslices(SQ):
                    ps = _proj_ps()
                    for ic in range(4):
                        nc.tensor.matmul(
                            ps[:, :qn],
                            t_wqT[:, ic, oc * 128:(oc + 1) * 128],
                            t_xqT[:, ic, qs:qs + qn],
                            start=(ic == 0),
                            stop=(ic == 3),
                        )
                    nc.vector.tensor_copy(t_QT[:, oc, qs:qs + qn], ps[:, :qn])

            def proj_k(oc):
                for ks, kn in nslices(SK):
                    ps = _proj_ps()
                    for ic in range(4):
                        nc.tensor.matmul(
                            ps[:, :kn],
                            t_wkT[:, ic, oc * 128:(oc + 1) * 128],
                            t_xkT[:, ic, ks:ks + kn],
                            start=(ic == 0),
                            stop=(ic == 3),
                        )
                    nc.vector.tensor_copy(t_KT[:, oc, ks:ks + kn], ps[:, :kn])

            proj_q(0)
            proj_k(0)
            for sc in range(KC):
                ps = _proj_ps()
                for ic in range(4):
                    nc.tensor.matmul(
                        ps[:],
                        t_xvT[:, ic, sc * 128:(sc + 1) * 128],
                        t_wvT[:, ic, :],
                        start=(ic == 0),
                        stop=(ic == 3),
                    )
                nc.vector.tensor_copy(
                    t_V[:, sc, :, 0:64],
                    ps.rearrange("p (h d) -> p h d", h=H),
                )
            for oc in range(1, 4):
                proj_q(oc)
                proj_k(oc)

            # ---- phase 2: attention (head pairs on PE row groups) ----
            for hc in range(4):
                ppv = {0: pv_pool.tile([128, SQ], f32, name="ppva", tag="ppva"),
                       1: pv_pool.tile([128, SQ], f32, name="ppvb", tag="ppvb")}
                for kc in range(KC):
                    es = {}
                    for hp in (0, 1):
                        pss = ps_pool.tile([128, SQ], f32, name="pss",
                                           tag="pssa" if hp == 0 else "pssb")
                        for qs, qn in nslices(SQ):
                            nc.tensor.matmul(
                                pss[:, qs:qs + qn],
                                t_KT[hp * 64:(hp + 1) * 64, hc,
                                     kc * 128:(kc + 1) * 128],
                                t_QT[hp * 64:(hp + 1) * 64, hc, qs:qs + qn],
                                start=True,
                                stop=True,
                            )
                        e = ep.tile([128, SQ], bf16, name="e",
                                    tag="ea" if hp == 0 else "eb")
                        nc.scalar.activation(
                            e[:], pss[:], Exp,
                            bias=t_bias[:, kc:kc + 1], scale=0.125,
                        )
                        es[hp] = e
                    for hp in (0, 1):
                        for qs, qn in nslices(SQ):
                            nc.tensor.matmul(
                                ppv[hp][:, qs:qs + qn],
                                t_V[:, kc, 2 * hc + hp, :],
                                es[hp][:, qs:qs + qn],
                                start=(kc == 0),
                                stop=(kc == KC - 1),
                            )
                for hp in (0, 1):
                    # Evacuate num+denom PSUM to SBUF in one copy (frees the
                    # PV accumulator for the next pair ASAP). DVE lanes are
                    # partition-locked and the custom-DVE reciprocal only
                    # works at base partition 0, so DMA the denominator from
                    # partitions 64:127 down to 0:63 before inverting.
                    pv_sb = rp.tile([128, SQ], f32, tag="pvsb")
                    den_lo = rp.tile([64, SQ], f32, tag="denlo")
                    rc_lo = rp.tile([64, SQ], f32, tag="rcl")
                    nc.vector.tensor_copy(pv_sb[:], ppv[hp][:])
                    nc.sync.dma_start(den_lo[:], pv_sb[64:128, :])
                    nc.vector.reciprocal_approx_fast(rc_lo[:], den_lo[:])
                    if hp == 0:
                        nc.vector.tensor_tensor(
                            t_OnT[0:64, hc, :], pv_sb[0:64, :], rc_lo[:], mult
                        )
                    else:
                        ot = rp.tile([64, SQ], bf16, tag="ottmp")
                        nc.vector.tensor_tensor(
                            ot[:], pv_sb[0:64, :], rc_lo[:], mult
                        )
                        nc.sync.dma_start(t_OnT[64:128, hc, :], ot[:])

            # ---- phase 3: output projection (reuses score psum slots) ----
            for qc in range(SQ // 128):
                po = _proj_ps()
                for c in range(4):
                    nc.tensor.matmul(
                        po[:],
                        t_OnT[:, c, qc * 128:(qc + 1) * 128],
                        t_woT[:, c, :],
                        start=(c == 0),
                        stop=(c == 3),
                    )
                ob = ob_pool.tile([128, 512], f32, tag="ob")
                nc.vector.tensor_copy(ob[:], po[:])
                nc.sync.dma_start(d_out[qc * 128:(qc + 1) * 128, :], ob[:])

    nc.finalize()
    return nc


def _pack_T(x):
    """[n, 512] fp32 -> transposed bf16 packed [128, 4, n] (contiguous)."""
    n = x.shape[0]
    return np.ascontiguousarray(
        x.T.astype(BF16).reshape(4, 128, n).transpose(1, 0, 2)
    )


def kernel(query, key, value, mask, W_q, W_k, W_v, W_o):
    global LAST_RESULTS, LAST_IN_MAPS
    from concourse.bass_utils import run_bass_kernel_spmd

    query = np.asarray(query, np.float32)
    key = np.asarray(key, np.float32)
    value = np.asarray(value, np.float32)
    mask = np.asarray(mask)

    # -- host prep: mask compaction, transposes, bf16 casts, packing --
    sels = [np.nonzero(mask[b, 0, 0] != 0)[0] for b in range(B)]
    SK = ((max(len(s) for s in sels) + 127) // 128) * 128
    KC = SK // 128

    per_batch = []
    for b in range(B):
        sel = sels[b]
        nk = len(sel)
        xk = np.zeros((SK, D), np.float32)
        xk[:nk] = key[b][sel]
        xv = np.zeros((SK, D), np.float32)
        xv[:nk] = value[b][sel]
        bias = np.full(SK, -30.0, np.float32)
        bias[:nk] = 0.0
        per_batch.append({
            "xkT": _pack_T(xk),
            "xvT": _pack_T(xv),
            "bias": np.ascontiguousarray(bias.reshape(KC, 128).T),
        })

    wqT = _pack_T(np.asarray(W_q, np.float32))   # W_q.T packed: [128,4,512]
    wkT = _pack_T(np.asarray(W_k, np.float32))
    wvT = _pack_T(np.asarray(W_v, np.float32))
    woT = _pack_T(np.asarray(W_o, np.float32))  # [128, 4, 512], head-pair rows

    in_maps = []
    for c in range(NCORES):
        b, qc = divmod(c, QSH)
        xq = query[b, qc * SQ:(qc + 1) * SQ]
        in_maps.append({
            "xqT": _pack_T(xq),
            "xkT": per_batch[b]["xkT"],
            "xvT": per_batch[b]["xvT"],
            "bias": per_batch[b]["bias"],
            "wqT": wqT, "wkT": wkT, "wvT": wvT, "woT": woT,
        })

    if KC not in _BUILD_CACHE:
        _BUILD_CACHE[KC] = _build(KC)
    nc = _BUILD_CACHE[KC]

    LAST_IN_MAPS = in_maps
    res = run_bass_kernel_spmd(nc, in_maps, core_ids=list(range(NCORES)))
    LAST_RESULTS = res

    out = np.empty((B, S, D), np.float32)
    for c in range(NCORES):
        b, qc = divmod(c, QSH)
        out[b, qc * SQ:(qc + 1) * SQ] = res.results[c]["out"]
    return out


# revision 25
# speedup vs baseline: 1.4969x; 1.2183x over previous
"""Trainium2 Bass kernel for nn_MultiHeadAttention (B=2, S=4096, D=512, H=8).

Sharding: sequence-parallel over queries. 8 cores = 2 batches x 4 query
quarters of 1024 rows each. Each core holds the full (mask-compacted) K/V
of its batch, computes its query rows end-to-end (Q/K/V projections,
masked softmax attention, output projection), and writes its disjoint
output rows. Host concatenates - no collectives needed.

Mask handling: the mask is a key-padding mask (per batch, per key).
Masked keys contribute exactly zero to softmax numerator and denominator,
so we compact them away on the host (halves all attention work; the
result is mathematically identical). Padding rows up to a multiple of
128 get a -30 additive bias so exp() sends them to ~1e-13.

Device dataflow (per core, SQ=1024 query rows, SK ~= 2176 keys):
  QT[o,q]  = WqT.T @ xqT     (bf16 matmuls, fp32 PSUM accumulation)
  KT[o,k]  = WkT.T @ xkT
  V[k,o]   = xvT.T @ WvT     -> packed as Vpad[k][h][V_h(64) | ones(64)]
  per head h:
    S^T[k,q] = KT_h.T(aka K_h) @ QT_h      (k-chunks of 128)
    E[k,q]   = exp(0.125*S^T + bias[k])    (ScalarE, bf16 out)
    PV[128,q] accum= Vpad_h.T @ E          rows 0:64 = numerator^T,
                                           rows 64:128 = denominator (x64)
    OnT[h]   = numerator^T * 1/denominator (bf16)
  out[q,j] = sum_h OnT_h.T @ WoT_h  (fp32) -> DMA to DRAM
"""

import numpy as np
import ml_dtypes

B, S, D, H, DK = 2, 4096, 512, 8, 64
NCORES = 8
QSH = 4          # query shards per batch
SQ = S // QSH    # 1024 query rows per core

BF16 = ml_dtypes.bfloat16

_BUILD_CACHE = {}
LAST_RESULTS = None
LAST_IN_MAPS = None


def _build(KC):
    """Build the Bass/Tile program for SK = KC*128 compacted+padded keys."""
    from contextlib import ExitStack

    import concourse.mybir as mybir
    import concourse.tile as tile
    from concourse import bacc

    SK = KC * 128
    f32 = mybir.dt.float32
    bf16 = mybir.dt.bfloat16

    nc = bacc.Bacc(
        "TRN2",
        target_bir_lowering=False,
        debug=False,
        enable_asserts=False,
        num_devices=NCORES,
    )

    def din(name, shape, dt):
        return nc.dram_tensor(name, shape, dt, kind="ExternalInput").ap()

    d_xqT = din("xqT", [128, 4, SQ], bf16)
    d_xkT = din("xkT", [128, 4, SK], bf16)
    d_xvT = din("xvT", [128, 4, SK], bf16)
    d_bias = din("bias", [128, KC], f32)
    d_wqT = din("wqT", [128, 4, D], bf16)
    d_wkT = din("wkT", [128, 4, D], bf16)
    d_wvT = din("wvT", [128, 4, D], bf16)
    d_woT = din("woT", [128, 4, D], bf16)
    d_out = nc.dram_tensor("out", [SQ, D], f32, kind="ExternalOutput").ap()

    Exp = mybir.ActivationFunctionType.Exp
    mult = mybir.AluOpType.mult

    def nslices(total, step=512):
        return [(s, min(step, total - s)) for s in range(0, total, step)]

    with tile.TileContext(nc) as tc:
        with ExitStack() as ctx:
            sb = ctx.enter_context(tc.tile_pool(name="sb", bufs=1))

            # ---- persistent SBUF tensors ----
            t_xqT = sb.tile([128, 4, SQ], bf16, tag="xqT")
            t_xkT = sb.tile([128, 4, SK], bf16, tag="xkT")
            t_xvT = sb.tile([128, 4, SK], bf16, tag="xvT")
            t_bias = sb.tile([128, KC], f32, tag="bias")
            t_wqT = sb.tile([128, 4, D], bf16, tag="wqT")
            t_wkT = sb.tile([128, 4, D], bf16, tag="wkT")
            t_wvT = sb.tile([128, 4, D], bf16, tag="wvT")
            t_woT = sb.tile([128, 4, D], bf16, tag="woT")
            t_QT = sb.tile([128, 4, SQ], bf16, tag="QT")
            t_KT = sb.tile([128, 4, SK], bf16, tag="KT")
            # Vpad[k, kc, h, 0:64] = V_h rows, [.., 64:128] = 1.0 (denominator)
            t_V = sb.tile([128, KC, H, 128], bf16, tag="V")
            # normalized attention out, head-PAIR packed: head 2c on
            # partitions 0:63, head 2c+1 on 64:127 (via DMA) -> K=128 final
            t_OnT = sb.tile([128, 4, SQ], bf16, tag="OnT")

            # Load order: first-needed first. A single dma_start tops out
            # around ~90 GB/s, so split the big tensors into per-ic chunks
            # across three DMA-issuing engines (SP/ACT HWDGE + Pool SWDGE;
            # ACT and Pool are idle at t=0) to run closer to HBM bandwidth.
            nc.sync.dma_start(t_wqT[:], d_wqT)
            nc.scalar.dma_start(t_wkT[:], d_wkT)
            nc.gpsimd.dma_start(t_wvT[:], d_wvT)
            for ic in range(4):
                nc.sync.dma_start(t_xqT[:, ic, :], d_xqT[:, ic, :])
            for ic in range(4):
                nc.scalar.dma_start(t_xkT[:, ic, :], d_xkT[:, ic, :])
                nc.sync.dma_start(t_xvT[:, ic, :], d_xvT[:, ic, :])
            nc.gpsimd.dma_start(t_bias[:], d_bias)
            nc.gpsimd.dma_start(t_woT[:], d_woT)

            nc.vector.memset(t_V[:, :, :, 64:128], 1.0)

            # PSUM budget (8 banks): head pairs run concurrently on PE
            # row-groups 0:63 / 64:127. Tags:
            #   pssa/pssb x1 buf = 4 banks (fp32 scores + projection psums)
            #   ppva/ppvb x1 buf = 4 banks (PV accumulators, fp32)
            ps_pool = ctx.enter_context(
                tc.tile_pool(name="ps_s", bufs=1, space="PSUM"))
            pv_pool = ctx.enter_context(
                tc.tile_pool(name="ps_pv", bufs=1, space="PSUM"))
            ep = ctx.enter_context(tc.tile_pool(name="ep", bufs=2))
            rp = ctx.enter_context(tc.tile_pool(name="rp", bufs=2))
            ob_pool = ctx.enter_context(tc.tile_pool(name="ob", bufs=3))

            # ---- phase 1: projections (ordered so heads 0/1 unblock first)
            _ptag = [0]

            def _proj_ps():
                _ptag[0] ^= 1
                return ps_pool.tile([128, 512], f32, name="psproj",
                                    tag="pssa" if _ptag[0] else "pssb")

            def proj_q(oc):
                for qs, qn in nslices(SQ):
                    ps = _proj_ps()
                    for ic in range(4):
                        nc.tensor.matmul(
                            ps[:, :qn],
                            t_wqT[:, ic, oc * 128:(oc + 1) * 128],
                            t_xqT[:, ic, qs:qs + qn],
                            start=(ic == 0),
                            stop=(ic == 3),
                        )
                    nc.vector.tensor_copy(t_QT[:, oc, qs:qs + qn], ps[:, :qn])

            def proj_k(oc):
                for ks, kn in nslices(SK):
                    ps = _proj_ps()
                    for ic in range(4):
                        nc.tensor.matmul(
                            ps[:, :kn],
                            t_wkT[:, ic, oc * 128:(oc + 1) * 128],
                            t_xkT[:, ic, ks:ks + kn],
                            start=(ic == 0),
                            stop=(ic == 3),
                        )
                    nc.vector.tensor_copy(t_KT[:, oc, ks:ks + kn], ps[:, :kn])

            proj_q(0)
            proj_k(0)
            for sc in range(KC):
                ps = _proj_ps()
                for ic in range(4):
                    nc.tensor.matmul(
                        ps[:],
                        t_xvT[:, ic, sc * 128:(sc + 1) * 128],
                        t_wvT[:, ic, :],
                        start=(ic == 0),
                        stop=(ic == 3),
                    )
                nc.vector.tensor_copy(
                    t_V[:, sc, :, 0:64],
                    ps.rearrange("p (h d) -> p h d", h=H),
                )
            for oc in range(1, 4):
                proj_q(oc)
                proj_k(oc)

            # ---- phase 2: attention (head pairs on PE row groups) ----
            for hc in range(4):
                ppv = {0: pv_pool.tile([128, SQ], f32, name="ppva", tag="ppva"),
                       1: pv_pool.tile([128, SQ], f32, name="ppvb", tag="ppvb")}
                for kc in range(KC):
                    es = {}
                    for hp in (0, 1):
                        pss = ps_pool.tile([128, SQ], f32, name="pss",
                                           tag="pssa" if hp == 0 else "pssb")
                        for qs, qn in nslices(SQ):
                            nc.tensor.matmul(
                                pss[:, qs:qs + qn],
                                t_KT[hp * 64:(hp + 1) * 64, hc,
                                     kc * 128:(kc + 1) * 128],
                                t_QT[hp * 64:(hp + 1) * 64, hc, qs:qs + qn],
                                start=True,
                                stop=True,
                            )
                        e = ep.tile([128, SQ], bf16, name="e",
                                    tag="ea" if hp == 0 else "eb")
                        nc.scalar.activation(
                            e[:], pss[:], Exp,
                            bias=t_bias[:, kc:kc + 1], scale=0.125,
                        )
                        es[hp] = e
                    for hp in (0, 1):
                        for qs, qn in nslices(SQ):
                            nc.tensor.matmul(
                                ppv[hp][:, qs:qs + qn],
                                t_V[:, kc, 2 * hc + hp, :],
                                es[hp][:, qs:qs + qn],
                                start=(kc == 0),
                                stop=(kc == KC - 1),
                            )
                for hp in (0, 1):
                    # Evacuate num+denom PSUM to SBUF in one copy (frees the
                    # PV accumulator for the next pair ASAP). DVE lanes are
                    # partition-locked and the custom-DVE reciprocal only
                    # works at base partition 0, so DMA the denominator from
                    # partitions 64:127 down to 0:63 before inverting.
                    pv_sb = rp.tile([128, SQ], f32, tag="pvsb")
                    den_lo = rp.tile([64, SQ], f32, tag="denlo")
                    rc_lo = rp.tile([64, SQ], f32, tag="rcl")
                    nc.vector.tensor_copy(pv_sb[:], ppv[hp][:])
                    nc.sync.dma_start(den_lo[:], pv_sb[64:128, :])
                    nc.vector.reciprocal_approx_fast(rc_lo[:], den_lo[:])
                    if hp == 0:
                        nc.vector.tensor_tensor(
                            t_OnT[0:64, hc, :], pv_sb[0:64, :], rc_lo[:], mult
                        )
                    else:
                        ot = rp.tile([64, SQ], bf16, tag="ottmp")
                        nc.vector.tensor_tensor(
                            ot[:], pv_sb[0:64, :], rc_lo[:], mult
                        )
                        nc.sync.dma_start(t_OnT[64:128, hc, :], ot[:])

            # ---- phase 3: output projection (reuses score psum slots) ----
            for qc in range(SQ // 128):
                po = _proj_ps()
                for c in range(4):
                    nc.tensor.matmul(
                        po[:],
                        t_OnT[:, c, qc * 128:(qc + 1) * 128],
                        t_woT[:, c, :],
                        start=(c == 0),
                        stop=(c == 3),
                    )
                ob = ob_pool.tile([128, 512], f32, tag="ob")
                nc.vector.tensor_copy(ob[:], po[:])
                nc.sync.dma_start(d_out[qc * 128:(qc + 1) * 128, :], ob[:])

    nc.finalize()
    return nc


def _pack_T(x):
    """[n, 512] fp32 -> transposed bf16 packed [128, 4, n] (contiguous)."""
    n = x.shape[0]
    return np.ascontiguousarray(
        x.T.astype(BF16).reshape(4, 128, n).transpose(1, 0, 2)
    )


def kernel(query, key, value, mask, W_q, W_k, W_v, W_o):
    global LAST_RESULTS, LAST_IN_MAPS
    from concourse.bass_utils import run_bass_kernel_spmd

    query = np.asarray(query, np.float32)
    key = np.asarray(key, np.float32)
    value = np.asarray(value, np.float32)
    mask = np.asarray(mask)

    # -- host prep: mask compaction, transposes, bf16 casts, packing --
    sels = [np.nonzero(mask[b, 0, 0] != 0)[0] for b in range(B)]
    SK = ((max(len(s) for s in sels) + 127) // 128) * 128
    KC = SK // 128

    per_batch = []
    for b in range(B):
        sel = sels[b]
        nk = len(sel)
        xk = np.zeros((SK, D), np.float32)
        xk[:nk] = key[b][sel]
        xv = np.zeros((SK, D), np.float32)
        xv[:nk] = value[b][sel]
        bias = np.full(SK, -30.0, np.float32)
        bias[:nk] = 0.0
        per_batch.append({
            "xkT": _pack_T(xk),
            "xvT": _pack_T(xv),
            "bias": np.ascontiguousarray(bias.reshape(KC, 128).T),
        })

    wqT = _pack_T(np.asarray(W_q, np.float32))   # W_q.T packed: [128,4,512]
    wkT = _pack_T(np.asarray(W_k, np.float32))
    wvT = _pack_T(np.asarray(W_v, np.float32))
    woT = _pack_T(np.asarray(W_o, np.float32))  # [128, 4, 512], head-pair rows

    in_maps = []
    for c in range(NCORES):
        b, qc = divmod(c, QSH)
        xq = query[b, qc * SQ:(qc + 1) * SQ]
        in_maps.append({
            "xqT": _pack_T(xq),
            "xkT": per_batch[b]["xkT"],
            "xvT": per_batch[b]["xvT"],
            "bias": per_batch[b]["bias"],
            "wqT": wqT, "wkT": wkT, "wvT": wvT, "woT": woT,
        })

    if KC not in _BUILD_CACHE:
        _BUILD_CACHE[KC] = _build(KC)
    nc = _BUILD_CACHE[KC]

    LAST_IN_MAPS = in_maps
    res = run_bass_kernel_spmd(nc, in_maps, core_ids=list(range(NCORES)))
    LAST_RESULTS = res

    out = np.empty((B, S, D), np.float32)
    for c in range(NCORES):
        b, qc = divmod(c, QSH)
        out[b, qc * SQ:(qc + 1) * SQ] = res.results[c]["out"]
    return out


# revision 26
# speedup vs baseline: 1.5110x; 1.0094x over previous
"""Trainium2 Bass kernel for nn_MultiHeadAttention (B=2, S=4096, D=512, H=8).

Sharding: sequence-parallel over queries. 8 cores = 2 batches x 4 query
quarters of 1024 rows each. Each core holds the full (mask-compacted) K/V
of its batch, computes its query rows end-to-end (Q/K/V projections,
masked softmax attention, output projection), and writes its disjoint
output rows. Host concatenates - no collectives needed.

Mask handling: the mask is a key-padding mask (per batch, per key).
Masked keys contribute exactly zero to softmax numerator and denominator,
so we compact them away on the host (halves all attention work; the
result is mathematically identical). Padding rows up to a multiple of
128 get a -30 additive bias so exp() sends them to ~1e-13.

Device dataflow (per core, SQ=1024 query rows, SK ~= 2176 keys):
  QT[o,q]  = WqT.T @ xqT     (bf16 matmuls, fp32 PSUM accumulation)
  KT[o,k]  = WkT.T @ xkT
  V[k,o]   = xvT.T @ WvT     -> packed as Vpad[k][h][V_h(64) | ones(64)]
  per head h:
    S^T[k,q] = KT_h.T(aka K_h) @ QT_h      (k-chunks of 128)
    E[k,q]   = exp(0.125*S^T + bias[k])    (ScalarE, bf16 out)
    PV[128,q] accum= Vpad_h.T @ E          rows 0:64 = numerator^T,
                                           rows 64:128 = denominator (x64)
    OnT[h]   = numerator^T * 1/denominator (bf16)
  out[q,j] = sum_h OnT_h.T @ WoT_h  (fp32) -> DMA to DRAM
"""

import numpy as np
import ml_dtypes

B, S, D, H, DK = 2, 4096, 512, 8, 64
NCORES = 8
QSH = 4          # query shards per batch
SQ = S // QSH    # 1024 query rows per core

BF16 = ml_dtypes.bfloat16

_BUILD_CACHE = {}
LAST_RESULTS = None
LAST_IN_MAPS = None


def _build(KC):
    """Build the Bass/Tile program for SK = KC*128 compacted+padded keys."""
    from contextlib import ExitStack

    import concourse.mybir as mybir
    import concourse.tile as tile
    from concourse import bacc

    SK = KC * 128
    f32 = mybir.dt.float32
    bf16 = mybir.dt.bfloat16

    nc = bacc.Bacc(
        "TRN2",
        target_bir_lowering=False,
        debug=False,
        enable_asserts=False,
        num_devices=NCORES,
    )

    def din(name, shape, dt):
        return nc.dram_tensor(name, shape, dt, kind="ExternalInput").ap()

    d_xqT = din("xqT", [128, 4, SQ], bf16)
    d_xkT = din("xkT", [128, 4, SK], bf16)
    d_xvT = din("xvT", [128, 4, SK], bf16)
    d_bias = din("bias", [128, KC], f32)
    d_wqT = din("wqT", [128, 4, D], bf16)
    d_wkT = din("wkT", [128, 4, D], bf16)
    d_wvT = din("wvT", [128, 4, D], bf16)
    d_woT = din("woT", [128, 4, D], bf16)
    d_out = nc.dram_tensor("out", [SQ, D], f32, kind="ExternalOutput").ap()

    Exp = mybir.ActivationFunctionType.Exp
    mult = mybir.AluOpType.mult

    def nslices(total, step=512):
        return [(s, min(step, total - s)) for s in range(0, total, step)]

    with tile.TileContext(nc) as tc:
        with ExitStack() as ctx:
            sb = ctx.enter_context(tc.tile_pool(name="sb", bufs=1))

            # ---- persistent SBUF tensors ----
            t_xqT = sb.tile([128, 4, SQ], bf16, tag="xqT")
            t_xkT = sb.tile([128, 4, SK], bf16, tag="xkT")
            t_xvT = sb.tile([128, 4, SK], bf16, tag="xvT")
            t_bias = sb.tile([128, KC], f32, tag="bias")
            t_wqT = sb.tile([128, 4, D], bf16, tag="wqT")
            t_wkT = sb.tile([128, 4, D], bf16, tag="wkT")
            t_wvT = sb.tile([128, 4, D], bf16, tag="wvT")
            t_woT = sb.tile([128, 4, D], bf16, tag="woT")
            t_QT = sb.tile([128, 4, SQ], bf16, tag="QT")
            t_KT = sb.tile([128, 4, SK], bf16, tag="KT")
            # Vpad[k, kc, h, 0:64] = V_h rows, [.., 64:128] = 1.0 (denominator)
            t_V = sb.tile([128, KC, H, 128], bf16, tag="V")
            # normalized attention out, head-PAIR packed: head 2c on
            # partitions 0:63, head 2c+1 on 64:127 (via DMA) -> K=128 final
            t_OnT = sb.tile([128, 4, SQ], bf16, tag="OnT")

            # Load order: first-needed first. A single dma_start tops out
            # around ~90 GB/s, so split the big tensors into per-ic chunks
            # across three DMA-issuing engines (SP/ACT HWDGE + Pool SWDGE;
            # ACT and Pool are idle at t=0) to run closer to HBM bandwidth.
            nc.sync.dma_start(t_wqT[:], d_wqT)
            nc.scalar.dma_start(t_wkT[:], d_wkT)
            nc.gpsimd.dma_start(t_wvT[:], d_wvT)
            for ic in range(4):
                nc.sync.dma_start(t_xqT[:, ic, :], d_xqT[:, ic, :])
            for ic in range(4):
                nc.scalar.dma_start(t_xkT[:, ic, :], d_xkT[:, ic, :])
                nc.sync.dma_start(t_xvT[:, ic, :], d_xvT[:, ic, :])
            nc.gpsimd.dma_start(t_bias[:], d_bias)
            nc.gpsimd.dma_start(t_woT[:], d_woT)

            nc.vector.memset(t_V[:, :, :, 64:128], 1.0)

            # PSUM budget (8 banks): head pairs run concurrently on PE
            # row-groups 0:63 / 64:127. Tags:
            #   pssa/pssb x1 buf = 4 banks (fp32 scores + projection psums)
            #   ppva/ppvb x1 buf = 4 banks (PV accumulators, fp32)
            ps_pool = ctx.enter_context(
                tc.tile_pool(name="ps_s", bufs=1, space="PSUM"))
            pv_pool = ctx.enter_context(
                tc.tile_pool(name="ps_pv", bufs=1, space="PSUM"))
            ep = ctx.enter_context(tc.tile_pool(name="ep", bufs=3))
            rp = ctx.enter_context(tc.tile_pool(name="rp", bufs=3))
            ob_pool = ctx.enter_context(tc.tile_pool(name="ob", bufs=3))

            # ---- phase 1: projections (ordered so heads 0/1 unblock first)
            _ptag = [0]

            def _proj_ps():
                _ptag[0] ^= 1
                return ps_pool.tile([128, 512], f32, name="psproj",
                                    tag="pssa" if _ptag[0] else "pssb")

            def proj_q(oc):
                for qs, qn in nslices(SQ):
                    ps = _proj_ps()
                    for ic in range(4):
                        nc.tensor.matmul(
                            ps[:, :qn],
                            t_wqT[:, ic, oc * 128:(oc + 1) * 128],
                            t_xqT[:, ic, qs:qs + qn],
                            start=(ic == 0),
                            stop=(ic == 3),
                        )
                    nc.vector.tensor_copy(t_QT[:, oc, qs:qs + qn], ps[:, :qn])

            def proj_k(oc):
                for ks, kn in nslices(SK):
                    ps = _proj_ps()
                    for ic in range(4):
                        nc.tensor.matmul(
                            ps[:, :kn],
                            t_wkT[:, ic, oc * 128:(oc + 1) * 128],
                            t_xkT[:, ic, ks:ks + kn],
                            start=(ic == 0),
                            stop=(ic == 3),
                        )
                    nc.vector.tensor_copy(t_KT[:, oc, ks:ks + kn], ps[:, :kn])

            proj_q(0)
            proj_k(0)
            for sc in range(KC):
                ps = _proj_ps()
                for ic in range(4):
                    nc.tensor.matmul(
                        ps[:],
                        t_xvT[:, ic, sc * 128:(sc + 1) * 128],
                        t_wvT[:, ic, :],
                        start=(ic == 0),
                        stop=(ic == 3),
                    )
                nc.vector.tensor_copy(
                    t_V[:, sc, :, 0:64],
                    ps.rearrange("p (h d) -> p h d", h=H),
                )
            for oc in range(1, 4):
                proj_q(oc)
                proj_k(oc)

            # ---- phase 2: attention (head pairs on PE row groups) ----
            for hc in range(4):
                ppv = {0: pv_pool.tile([128, SQ], f32, name="ppva", tag="ppva"),
                       1: pv_pool.tile([128, SQ], f32, name="ppvb", tag="ppvb")}
                for kc in range(KC):
                    es = {}
                    for hp in (0, 1):
                        pss = ps_pool.tile([128, SQ], f32, name="pss",
                                           tag="pssa" if hp == 0 else "pssb")
                        for qs, qn in nslices(SQ):
                            nc.tensor.matmul(
                                pss[:, qs:qs + qn],
                                t_KT[hp * 64:(hp + 1) * 64, hc,
                                     kc * 128:(kc + 1) * 128],
                                t_QT[hp * 64:(hp + 1) * 64, hc, qs:qs + qn],
                                start=True,
                                stop=True,
                            )
                        e = ep.tile([128, SQ], bf16, name="e",
                                    tag="ea" if hp == 0 else "eb")
                        nc.scalar.activation(
                            e[:], pss[:], Exp,
                            bias=t_bias[:, kc:kc + 1], scale=0.125,
                        )
                        es[hp] = e
                    for hp in (0, 1):
                        for qs, qn in nslices(SQ):
                            nc.tensor.matmul(
                                ppv[hp][:, qs:qs + qn],
                                t_V[:, kc, 2 * hc + hp, :],
                                es[hp][:, qs:qs + qn],
                                start=(kc == 0),
                                stop=(kc == KC - 1),
                            )
                for hp in (0, 1):
                    # Evacuate num+denom PSUM to SBUF in one copy (frees the
                    # PV accumulator for the next pair ASAP). DVE lanes are
                    # partition-locked and the custom-DVE reciprocal only
                    # works at base partition 0, so DMA the denominator from
                    # partitions 64:127 down to 0:63 before inverting.
                    pv_sb = rp.tile([128, SQ], f32, tag="pvsb")
                    den_lo = rp.tile([64, SQ], f32, tag="denlo")
                    rc_lo = rp.tile([64, SQ], f32, tag="rcl")
                    nc.vector.tensor_copy(pv_sb[:], ppv[hp][:])
                    nc.sync.dma_start(den_lo[:], pv_sb[64:128, :])
                    nc.vector.reciprocal_approx_fast(rc_lo[:], den_lo[:])
                    if hp == 0:
                        nc.vector.tensor_tensor(
                            t_OnT[0:64, hc, :], pv_sb[0:64, :], rc_lo[:], mult
                        )
                    else:
                        ot = rp.tile([64, SQ], bf16, tag="ottmp")
                        nc.vector.tensor_tensor(
                            ot[:], pv_sb[0:64, :], rc_lo[:], mult
                        )
                        nc.sync.dma_start(t_OnT[64:128, hc, :], ot[:])

            # ---- phase 3: output projection (reuses score psum slots) ----
            for qc in range(SQ // 128):
                po = _proj_ps()
                for c in range(4):
                    nc.tensor.matmul(
                        po[:],
                        t_OnT[:, c, qc * 128:(qc + 1) * 128],
                        t_woT[:, c, :],
                        start=(c == 0),
                        stop=(c == 3),
                    )
                ob = ob_pool.tile([128, 512], f32, tag="ob")
                nc.vector.tensor_copy(ob[:], po[:])
                nc.sync.dma_start(d_out[qc * 128:(qc + 1) * 128, :], ob[:])

    nc.finalize()
    return nc


def _pack_T(x):
    """[n, 512] fp32 -> transposed bf16 packed [128, 4, n] (contiguous)."""
    n = x.shape[0]
    return np.ascontiguousarray(
        x.T.astype(BF16).reshape(4, 128, n).transpose(1, 0, 2)
    )


def kernel(query, key, value, mask, W_q, W_k, W_v, W_o):
    global LAST_RESULTS, LAST_IN_MAPS
    from concourse.bass_utils import run_bass_kernel_spmd

    query = np.asarray(query, np.float32)
    key = np.asarray(key, np.float32)
    value = np.asarray(value, np.float32)
    mask = np.asarray(mask)

    # -- host prep: mask compaction, transposes, bf16 casts, packing --
    sels = [np.nonzero(mask[b, 0, 0] != 0)[0] for b in range(B)]
    SK = ((max(len(s) for s in sels) + 127) // 128) * 128
    KC = SK // 128

    per_batch = []
    for b in range(B):
        sel = sels[b]
        nk = len(sel)
        xk = np.zeros((SK, D), np.float32)
        xk[:nk] = key[b][sel]
        xv = np.zeros((SK, D), np.float32)
        xv[:nk] = value[b][sel]
        bias = np.full(SK, -30.0, np.float32)
        bias[:nk] = 0.0
        per_batch.append({
            "xkT": _pack_T(xk),
            "xvT": _pack_T(xv),
            "bias": np.ascontiguousarray(bias.reshape(KC, 128).T),
        })

    wqT = _pack_T(np.asarray(W_q, np.float32))   # W_q.T packed: [128,4,512]
    wkT = _pack_T(np.asarray(W_k, np.float32))
    wvT = _pack_T(np.asarray(W_v, np.float32))
    woT = _pack_T(np.asarray(W_o, np.float32))  # [128, 4, 512], head-pair rows

    in_maps = []
    for c in range(NCORES):
        b, qc = divmod(c, QSH)
        xq = query[b, qc * SQ:(qc + 1) * SQ]
        in_maps.append({
            "xqT": _pack_T(xq),
            "xkT": per_batch[b]["xkT"],
            "xvT": per_batch[b]["xvT"],
            "bias": per_batch[b]["bias"],
            "wqT": wqT, "wkT": wkT, "wvT": wvT, "woT": woT,
        })

    if KC not in _BUILD_CACHE:
        _BUILD_CACHE[KC] = _build(KC)
    nc = _BUILD_CACHE[KC]

    LAST_IN_MAPS = in_maps
    res = run_bass_kernel_spmd(nc, in_maps, core_ids=list(range(NCORES)))
    LAST_RESULTS = res

    out = np.empty((B, S, D), np.float32)
    for c in range(NCORES):
        b, qc = divmod(c, QSH)
        out[b, qc * SQ:(qc + 1) * SQ] = res.results[c]["out"]
    return out
